# revision 1
# baseline (speedup 1.0000x reference)
"""COMPOLBlock2d kernel: data-parallel over batch B=8 across 8 trn2 NeuronCores.

FFTs are expressed as DFT matmuls (real/imag split) so the whole graph lowers
to tensor-engine matmuls + pointwise ops; no fft primitives on device.
"""
import numpy as np
import jax
import jax.numpy as jnp
from functools import partial

B, C, H, W = 8, 64, 128, 128
M, M1, M2 = 4, 16, 16

# ---- DFT matrices (host, fp32) ----
_rows = np.concatenate([np.arange(M1), np.arange(H - M1, H)])  # 32 k1 rows
_h = np.arange(H)
_w = np.arange(W)
_k2 = np.arange(M2)
# forward: xf_sel[k1,k2] = sum_hw x[h,w] e^{-2pi i (k1 h/H + k2 w/W)}
_Fh = np.exp(-2j * np.pi * np.outer(_rows, _h) / H)          # [32,H]
_Fw = np.exp(-2j * np.pi * np.outer(_k2, _w) / W)            # [16,W]
FH_RE = np.ascontiguousarray(_Fh.real, dtype=np.float32)
FH_IM = np.ascontiguousarray(_Fh.imag, dtype=np.float32)
FW_RE = np.ascontiguousarray(_Fw.real, dtype=np.float32)
FW_IM = np.ascontiguousarray(_Fw.imag, dtype=np.float32)
# inverse H: Y[h] = (1/H) sum_k1 O[k1] e^{+2pi i k1 h/H}
_Gh = np.exp(2j * np.pi * np.outer(_h, _rows) / H) / H       # [H,32]
GH_RE = np.ascontiguousarray(_Gh.real, dtype=np.float32)
GH_IM = np.ascontiguousarray(_Gh.imag, dtype=np.float32)
# inverse W (irfft semantics: imag of bin 0 discarded, bins 1..15 doubled)
_c = np.where(_k2 == 0, 1.0, 2.0)[:, None]
CW = np.ascontiguousarray(_c * np.cos(2 * np.pi * np.outer(_k2, _w) / W) / W,
                          dtype=np.float32)                   # [16,W]
SW = np.ascontiguousarray(_c * np.sin(2 * np.pi * np.outer(_k2, _w) / W) / W,
                          dtype=np.float32)                   # [16,W]

_PREC = jax.lax.Precision.HIGHEST


def _ein(s, *ops):
    return jnp.einsum(s, *ops, precision=_PREC)


def _atn(x0, x1, x2, x3, c_w, c_b, wq_w, wq_b, wk_w, wk_b, wa_w, wa_b):
    V = jnp.stack([x0, x1, x2, x3], axis=1)         # [b,M,C,H,W]
    V = jnp.transpose(V, (0, 3, 4, 1, 2))           # [b,H,W,M,C]
    Vc = _ein('bhwmc,oc->bhwmo', V, c_w) + c_b
    Q = _ein('bhwc,oc->bhwo', jnp.mean(Vc, axis=-2), wq_w) + wq_b  # [b,H,W,C]
    K = _ein('bhwmc,oc->bhwmo', Vc, wk_w) + wk_b
    A = _ein('bhwmc,oc->bhwmo', Vc, wa_w) + wa_b
    scores = _ein('bhwc,bhwmc->bhwm', Q, K) / jnp.sqrt(jnp.float32(C))
    alpha = jax.nn.softmax(scores, axis=-1)         # [b,H,W,M]
    z = _ein('bhwm,bhwmc->bhwc', alpha, A)
    return jnp.transpose(z, (0, 3, 1, 2))           # [b,C,H,W]


def _spectral(xs, w1_re, w1_im, w2_re, w2_im):
    # xs: [M,b,C,H,W] real. forward partial DFT -> [M,b,C,32,16] complex
    t_re = _ein('mbchw,kh->mbckw', xs, FH_RE)
    t_im = _ein('mbchw,kh->mbckw', xs, FH_IM)
    xf_re = _ein('mbckw,jw->mbckj', t_re, FW_RE) - _ein('mbckw,jw->mbckj', t_im, FW_IM)
    xf_im = _ein('mbckw,jw->mbckj', t_re, FW_IM) + _ein('mbckw,jw->mbckj', t_im, FW_RE)
    # mode mix: low rows (k<M1) with w1, high rows with w2
    lo_re, lo_im = xf_re[:, :, :, :M1], xf_im[:, :, :, :M1]
    hi_re, hi_im = xf_re[:, :, :, M1:], xf_im[:, :, :, M1:]
    o1_re = _ein('mbixy,mioxy->mboxy', lo_re, w1_re) - _ein('mbixy,mioxy->mboxy', lo_im, w1_im)
    o1_im = _ein('mbixy,mioxy->mboxy', lo_re, w1_im) + _ein('mbixy,mioxy->mboxy', lo_im, w1_re)
    o2_re = _ein('mbixy,mioxy->mboxy', hi_re, w2_re) - _ein('mbixy,mioxy->mboxy', hi_im, w2_im)
    o2_im = _ein('mbixy,mioxy->mboxy', hi_re, w2_im) + _ein('mbixy,mioxy->mboxy', hi_im, w2_re)
    O_re = jnp.concatenate([o1_re, o2_re], axis=3)  # [M,b,C,32,16]
    O_im = jnp.concatenate([o1_im, o2_im], axis=3)
    # inverse H
    Y_re = _ein('mbokj,hk->mbohj', O_re, GH_RE) - _ein('mbokj,hk->mbohj', O_im, GH_IM)
    Y_im = _ein('mbokj,hk->mbohj', O_re, GH_IM) + _ein('mbokj,hk->mbohj', O_im, GH_RE)
    # inverse W (irfft)
    return _ein('mbohj,jw->mbohw', Y_re, CW) - _ein('mbohj,jw->mbohw', Y_im, SW)


def _block(x0, x1, x2, x3, c_w, c_b, wq_w, wq_b, wk_w, wk_b, wa_w, wa_b,
           w_w, w_b, wz_w, wz_b, sw1_re, sw1_im, sw2_re, sw2_im):
    z = _atn(x0, x1, x2, x3, c_w, c_b, wq_w, wq_b, wk_w, wk_b, wa_w, wa_b)
    xs = jnp.stack([x0, x1, x2, x3], axis=0)        # [M,b,C,H,W]
    sp = _spectral(xs, sw1_re, sw1_im, sw2_re, sw2_im)
    wx = _ein('mbihw,moi->mbohw', xs, w_w) + w_b[:, None, :, None, None]
    wzz = _ein('bihw,moi->mbohw', z, wz_w) + wz_b[:, None, :, None, None]
    xs_new = jax.nn.gelu(sp + wx + wzz, approximate=False)
    return xs_new, z


_pblock = jax.pmap(_block, in_axes=((0, 0, 0, 0) + (None,) * 16))


def kernel(x0, x1, x2, x3, c_w, c_b, wq_w, wq_b, wk_w, wk_b, wa_w, wa_b,
           w_w, w_b, wz_w, wz_b, sw1_re, sw1_im, sw2_re, sw2_im):
    nd = min(8, jax.device_count())
    assert nd == 8, f"expected 8 cores, got {nd}"
    shard = lambda a: a.reshape(nd, B // nd, *a.shape[1:])
    xs_new, z = _pblock(shard(x0), shard(x1), shard(x2), shard(x3),
                        c_w, c_b, wq_w, wq_b, wk_w, wk_b, wa_w, wa_b,
                        w_w, w_b, wz_w, wz_b, sw1_re, sw1_im, sw2_re, sw2_im)
    xs_new = np.asarray(xs_new)   # [nd, M, B/nd, C, H, W]
    z = np.asarray(z)             # [nd, B/nd, C, H, W]
    xs_new = np.moveaxis(xs_new, 0, 1).reshape(M, B, C, H, W)
    z = z.reshape(B, C, H, W)
    return xs_new, z
